# revision 23
# baseline (speedup 1.0000x reference)
"""Chamfer distance kernel for Trainium2 (8 NeuronCores, SPMD).

Problem: x, y ~ [4, 8192, 3] f32.  Output: scalar f32
    mean_i min_j ||x_i - y_j||^2  +  mean_j min_i ||x_i - y_j||^2
(means over batch*8192).

Strategy
--------
Shard (batch b, x-row half h) across the 8 cores: core c = 2*b + h owns
x[b, h*4096:(h+1)*4096] vs all of y[b].

Each core computes the 4096x8192 block of NEGATED squared distances with
bf16 matmuls:
    -dist[i,j] = -x2[i] - y2[j] + 2*sum_d x[i,d]*y[j,d]
where every f32 factor is split into bf16 hi+lo parts (4 K-rows for the
norm terms, 12 for the 3 coordinate products) so precision is ~fp32 while
the PE runs at bf16 speed.  K is zero-padded 16 -> 128: zero rows add
nothing numerically, cost no extra PE cycles (stream time is column
count), let the inputs ride full-bandwidth 128-partition DMAs, and keep
the PE activity monitor from seeing a mostly-idle array.

Negation turns both min-reductions into max-reductions.  Post-matmul,
per row-block ib (128 x-rows vs all 8192 y):
  * ACT evacuates the four [128,2048] PSUM tiles into one contiguous
    fp16 st_super [128, 8192] in SBUF (ACT is the only fast PSUM reader).
  * DVE col direction: ONE wide tensor_tensor max of st_super into
    colacc [128, 8192] (fp16, 2x mode).
  * DVE row direction: binary fold tree straight off st_super
    (4096/2048/1024/512-wide fp16 TT maxes at 2x, then one 512-wide
    tensor_reduce) -> rowcol[:, ib].  Folding directly is cheaper than
    running a per-chunk accumulator and folding it afterwards.
The final cross-partition max of colacc is done on the HOST (colacc is
DMA'd out as fp16 [128, 8192]), which removes the gpsimd tail entirely.

Host combines: row maxes are complete per core (sum them); column partial
maxes [128, 8192] are maxed over partitions and between the two cores
sharing a batch, then summed.  All final means in f64, returned as f32.
"""

import numpy as np
import ml_dtypes

import concourse.bacc as bacc
import concourse.bass as bass
import concourse.mybir as mybir
import concourse.tile as tile
from concourse.bass_utils import run_bass_kernel_spmd

BF16 = ml_dtypes.bfloat16

B = 4
N = 8192
D = 3
HALF = N // 2           # x-rows per core
NCORES = 8
KRAW = 16               # real augmented contraction rows
K = 128                 # zero-padded contraction dim
MBLK = 128              # x-rows per block (PSUM partition dim)
NBLKS = HALF // MBLK    # 32
PSUM_W = 2048           # psum tile width (4 f32 banks)
NPS = N // PSUM_W       # 4 psum tiles per row block
MM_N = 512              # matmul free width (1 f32 psum bank)

_NC_CACHE = None


def _build_nc():
    global _NC_CACHE
    if _NC_CACHE is not None:
        return _NC_CACHE

    nc = bacc.Bacc("TRN2", target_bir_lowering=False, debug=False,
                   num_devices=NCORES)
    lhs_d = nc.dram_tensor("lhst", [K, HALF], mybir.dt.bfloat16,
                           kind="ExternalInput")
    rhs_d = nc.dram_tensor("rhs", [K, N], mybir.dt.bfloat16,
                           kind="ExternalInput")
    row_d = nc.dram_tensor("rowout", [MBLK, NBLKS], mybir.dt.float32,
                           kind="ExternalOutput")
    col_d = nc.dram_tensor("colout", [MBLK, N], mybir.dt.float16,
                           kind="ExternalOutput")

    with tile.TileContext(nc) as tc:
        with tc.tile_pool(name="sb", bufs=1) as sb, \
             tc.tile_pool(name="ps", bufs=2, space=bass.MemorySpace.PSUM) as ps:
            lhs_sb = sb.tile([K, HALF], mybir.dt.bfloat16)
            rhs_sb = sb.tile([K, N], mybir.dt.bfloat16)
            # chunked input DMAs so the first matmuls' operands land early
            nc.sync.dma_start(lhs_sb[:, 0:MBLK], lhs_d.ap()[:, 0:MBLK])
            nc.sync.dma_start(rhs_sb[:, 0:MM_N], rhs_d.ap()[:, 0:MM_N])
            nc.sync.dma_start(rhs_sb[:, MM_N:PSUM_W], rhs_d.ap()[:, MM_N:PSUM_W])
            for q in range(1, NPS):
                qs, qe = q * PSUM_W, (q + 1) * PSUM_W
                nc.sync.dma_start(rhs_sb[:, qs:qe], rhs_d.ap()[:, qs:qe])
            nc.sync.dma_start(lhs_sb[:, MBLK:HALF], lhs_d.ap()[:, MBLK:HALF])

            colacc = sb.tile([MBLK, N], mybir.dt.float16)
            rowcol = sb.tile([MBLK, NBLKS], mybir.dt.float32)
            # per-ib 512-wide row-max partials; the narrow fold levels are
            # batched across all 32 ibs at the end (strided 3-D APs) so the
            # per-op overhead is paid once per level, not once per ib.
            rowacc = sb.tile([MBLK, NBLKS, 512], mybir.dt.float16)

            # 4-slot fold scratch: four ibs' half-folded rows, so the
            # narrower fold levels run once per quad as strided 3-D ops
            # (one 58-cycle op overhead per level instead of four).
            scrq = sb.tile([MBLK, 4, N // 2], mybir.dt.float16)

            with tc.tile_pool(name="stage", bufs=3) as stg:
                for ib in range(NBLKS):
                    w = lhs_sb[:, ib * MBLK:(ib + 1) * MBLK]
                    last = ib == NBLKS - 1
                    sts = stg.tile([MBLK, N], mybir.dt.float16, tag="stage")
                    for jp in range(NPS):
                        pt = ps.tile([MBLK, PSUM_W], mybir.dt.float32)
                        for m in range(PSUM_W // MM_N):
                            j0 = jp * PSUM_W + m * MM_N
                            nc.tensor.matmul(pt[:, m * MM_N:(m + 1) * MM_N],
                                             w, rhs_sb[:, j0:j0 + MM_N],
                                             start=True, stop=True)
                        nc.scalar.copy(
                            out=sts[:, jp * PSUM_W:(jp + 1) * PSUM_W],
                            in_=pt[:])
                        # column direction: per-quarter at ib 0 (starts DVE
                        # as soon as the first quarter lands) and at the
                        # last ib (lets each quarter's output DMA overlap).
                        qs, qe = jp * PSUM_W, (jp + 1) * PSUM_W
                        if ib == 0:
                            nc.vector.tensor_copy(out=colacc[:, qs:qe],
                                                  in_=sts[:, qs:qe])
                        elif last:
                            nc.vector.tensor_tensor(
                                out=colacc[:, qs:qe], in0=colacc[:, qs:qe],
                                in1=sts[:, qs:qe], op=mybir.AluOpType.max)
                            nc.sync.dma_start(col_d.ap()[:, qs:qe],
                                              colacc[:, qs:qe])
                    # column direction: one wide running max over middle ibs
                    if 0 < ib < NBLKS - 1:
                        nc.vector.tensor_tensor(out=colacc[:], in0=colacc[:],
                                                in1=sts[:],
                                                op=mybir.AluOpType.max)
                    # row direction: per-ib first fold into a quad slot
                    nc.vector.tensor_tensor(out=scrq[:, ib % 4, :],
                                            in0=sts[:, 0:N // 2],
                                            in1=sts[:, N // 2:N],
                                            op=mybir.AluOpType.max)
                    if ib % 4 == 3:
                        # narrower levels for 4 ibs at once (strided 3-D)
                        for wdt in (2048, 1024):
                            nc.vector.tensor_tensor(
                                out=scrq[:, :, 0:wdt],
                                in0=scrq[:, :, 0:wdt],
                                in1=scrq[:, :, wdt:2 * wdt],
                                op=mybir.AluOpType.max)
                        nc.vector.tensor_tensor(
                            out=rowacc[:, ib - 3:ib + 1, :],
                            in0=scrq[:, :, 0:512],
                            in1=scrq[:, :, 512:1024],
                            op=mybir.AluOpType.max)

            # batched narrow folds: all 32 row blocks at once
            wdt = 256
            while wdt >= 2:
                nc.vector.tensor_tensor(out=rowacc[:, :, 0:wdt],
                                        in0=rowacc[:, :, 0:wdt],
                                        in1=rowacc[:, :, wdt:2 * wdt],
                                        op=mybir.AluOpType.max)
                wdt //= 2
            nc.vector.tensor_reduce(
                out=rowcol[:], in_=rowacc[:, :, 0:2],
                axis=mybir.AxisListType.X, op=mybir.AluOpType.max)
            nc.sync.dma_start(row_d.ap()[:], rowcol[:])

    nc.compile()
    _NC_CACHE = nc
    return nc


def _split(v):
    """f32 -> (hi, lo) bf16 with v ~= hi + lo to ~18 mantissa bits."""
    hi = v.astype(BF16)
    lo = (v - hi.astype(np.float32)).astype(BF16)
    return hi, lo


def _prep_core(xc, yb):
    """Build the K=128 (16 real + 112 zero) augmented bf16 operands for one
    core.

    xc: [HALF, 3] f32 x-chunk; yb: [N, 3] f32 full y for the batch.
    Row k of lhs/rhs multiply pairwise and accumulate so that
    psum[i, j] = -dist2(x_i, y_j) to ~1e-5 abs.
    """
    x2 = np.sum(xc * xc, axis=1)
    y2 = np.sum(yb * yb, axis=1)
    nx2h, nx2l = _split(-x2)
    ny2h, ny2l = _split(-y2)
    xh, xl = _split(xc)
    yh, yl = _split(yb)
    txh = (xh.astype(np.float32) * 2.0).astype(BF16)  # exact in bf16
    txl = (xl.astype(np.float32) * 2.0).astype(BF16)

    lhs = np.zeros((K, HALF), dtype=BF16)
    rhs = np.zeros((K, N), dtype=BF16)
    lhs[0] = nx2h
    lhs[1] = nx2l
    rhs[0] = np.ones(N, BF16)
    rhs[1] = np.ones(N, BF16)
    lhs[2] = np.ones(HALF, BF16)
    lhs[3] = np.ones(HALF, BF16)
    rhs[2] = ny2h
    rhs[3] = ny2l
    for d in range(D):
        lhs[4 + d] = txh[:, d]
        rhs[4 + d] = yh[:, d]
        lhs[7 + d] = txl[:, d]
        rhs[7 + d] = yh[:, d]
        lhs[10 + d] = txh[:, d]
        rhs[10 + d] = yl[:, d]
        lhs[13 + d] = txl[:, d]
        rhs[13 + d] = yl[:, d]
    return {"lhst": lhs, "rhs": rhs}


def make_in_maps(x, y):
    x = np.ascontiguousarray(np.asarray(x, dtype=np.float32))
    y = np.ascontiguousarray(np.asarray(y, dtype=np.float32))
    in_maps = []
    for c in range(NCORES):
        b, h = c // 2, c % 2
        in_maps.append(_prep_core(x[b, h * HALF:(h + 1) * HALF], y[b]))
    return in_maps


def combine(results):
    """results: list of 8 dicts with 'rowout' [128, 32] and
    'colout' [128, N] fp16, both holding NEGATED maxes (-min distances).
    colout still needs its cross-partition max (done here on host)."""
    row_sum = 0.0
    col_sum = 0.0
    for b in range(B):
        r0 = results[2 * b]["rowout"].astype(np.float64).sum()
        r1 = results[2 * b + 1]["rowout"].astype(np.float64).sum()
        row_sum += -(r0 + r1)
        c0 = results[2 * b]["colout"].astype(np.float32).max(axis=0)
        c1 = results[2 * b + 1]["colout"].astype(np.float32).max(axis=0)
        col_sum += -np.maximum(c0, c1).astype(np.float64).sum()
    mean_x = row_sum / (B * N)
    mean_y = col_sum / (B * N)
    return np.asarray(mean_x + mean_y, dtype=np.float32)


def kernel(x, y):
    nc = _build_nc()
    in_maps = make_in_maps(x, y)
    res = run_bass_kernel_spmd(nc, in_maps, core_ids=list(range(NCORES)))
    return combine(res.results)
